# revision 2
# baseline (speedup 1.0000x reference)
"""Trainium2 Bass kernel for nn_NNFFTLayer (radix-R butterfly mix layer).

Reference computation (per position p, last dim N=8192):
    scale = tile(weights, R)                  # weights: [1024], R=8 -> [8192]
    y     = (scale * x).reshape(..., 64, 8, 16)   # [k, i, c]
    out[..., k, j, c] = sum_i lin_weights[j, i] * y[..., k, i, c]

Each 128-element chunk k of the last dim undergoes an independent linear map
M_km (km = k % 8) that folds the scale and the 8x8 mix:
    M_km[j*16+c', i*16+c] = L[j,i] * weights[km*128 + i*16 + c] * (c' == c)

Device strategy (pure data parallel over 8 cores, 1024 positions each):
  - host casts x (and the 128x128 chunk matrices) to bf16: the kernel is
    purely HBM-bandwidth bound, so halving the bytes halves the runtime;
    bf16 rounding costs ~0.3% relative error, well under the 2e-2 gate
  - DMA contiguous bf16 slabs x[128 pos, 8192] into SBUF
  - per 128-wide chunk: PE transpose (bf16 in -> bf16 out in PSUM), DVE
    copy -> SBUF, PE bf16 matmul against the precomputed 128x128 constant
    (rhs resident in SBUF) -> f32 PSUM, ACT copy downcast -> bf16 out slab
  - DMA slab back (bf16), host upcasts to f32.
  Memory-bound: ~16 MiB in + 16 MiB out per core.
"""

import sys

if "/opt/trn_rl_repo" not in sys.path:
    sys.path.insert(0, "/opt/trn_rl_repo")

import numpy as np
import ml_dtypes

BF16 = ml_dtypes.bfloat16

P = 128
N = 8192
R = 8
TWO_R = 16
N_CHUNKS = N // P        # 64
KM = 1024 // P           # 8 distinct per-chunk matrices
N_CORES = 8
POS_TOTAL = 4 * 2048     # 8192 positions (batch*seq)
POS_PER_CORE = POS_TOTAL // N_CORES   # 1024
SLABS = POS_PER_CORE // P             # 8

_CACHE = {}


def _build_nc():
    import concourse.bacc as bacc
    import concourse.mybir as mybir
    import concourse.tile as tile

    nc = bacc.Bacc("TRN2", target_bir_lowering=False, debug=False)
    f32 = mybir.dt.float32
    bf16 = mybir.dt.bfloat16
    xs = nc.dram_tensor("xs", (POS_PER_CORE, N), bf16, kind="ExternalInput")
    mt = nc.dram_tensor("mt", (P, KM * P), bf16, kind="ExternalInput")
    idt = nc.dram_tensor("idt", (P, P), bf16, kind="ExternalInput")
    out = nc.dram_tensor("out", (POS_PER_CORE, N), bf16, kind="ExternalOutput")

    G = 4               # chunks per PSUM bank / per batched copy
    N_GROUPS = N_CHUNKS // G   # 16 groups per slab
    PIECES = 4          # DMA pieces per slab (512 KiB each)
    PW = N // PIECES

    with tile.TileContext(nc) as tc:
        with (
            tc.tile_pool(name="singles", bufs=1) as singles,
            tc.tile_pool(name="xin", bufs=2) as xin,
            tc.tile_pool(name="outp", bufs=2) as outp,
            tc.tile_pool(name="xt", bufs=4) as xtp,
            tc.tile_pool(name="tp_ps", bufs=4, space="PSUM") as tp_ps,
            tc.tile_pool(name="mm_ps", bufs=4, space="PSUM") as mm_ps,
        ):
            # tiny ident first on the SP ring (lands earliest, feeds the PE
            # warmup); mt on the ACT ring behind the activation table load
            ident = singles.tile([P, P], bf16)
            nc.sync.dma_start(ident[:], idt[:, :])
            mt_sb = singles.tile([P, KM * P], bf16)
            nc.scalar.dma_start(mt_sb[:], mt[:, :])

            for s in range(SLABS):
                xsb = xin.tile([P, N], bf16)
                # fine pieces on the first slab so PE starts sooner; one
                # big transfer elsewhere (loads run a slab ahead, so only
                # DMA throughput matters mid-stream)
                pieces = PIECES * 2 if s == 0 else PIECES
                pw = N // pieces
                for p in range(pieces):
                    nc.sync.dma_start(
                        xsb[:, p * pw:(p + 1) * pw],
                        xs[s * P:(s + 1) * P, p * pw:(p + 1) * pw],
                    )
                osb = outp.tile([P, N], bf16)
                for g in range(N_GROUPS):
                    pst = tp_ps.tile([P, G * P], bf16)
                    for q in range(G):
                        k = g * G + q
                        nc.tensor.transpose(
                            pst[:, q * P:(q + 1) * P],
                            xsb[:, k * P:(k + 1) * P], ident[:],
                        )
                    xt = xtp.tile([P, G * P], bf16)
                    nc.vector.tensor_copy(xt[:], pst[:])
                    mm = mm_ps.tile([P, G * P], f32)
                    for q in range(G):
                        km = (g * G + q) % KM
                        nc.tensor.matmul(
                            mm[:, q * P:(q + 1) * P],
                            lhsT=xt[:, q * P:(q + 1) * P],
                            rhs=mt_sb[:, km * P:(km + 1) * P],
                            start=True, stop=True,
                        )
                    nc.scalar.copy(osb[:, g * G * P:(g + 1) * G * P], mm[:])
                spieces = PIECES
                spw = N // spieces
                # last slab drains on the SP ring, which is idle by then
                seng = nc.sync if s == SLABS - 1 else nc.scalar
                for p in range(spieces):
                    seng.dma_start(
                        out[s * P:(s + 1) * P, p * spw:(p + 1) * spw],
                        osb[:, p * spw:(p + 1) * spw],
                    )

    # Strip the framework's const-register memsets from the entry block:
    # they are unused here, but their GpSimd library load (~6us Q7 boot)
    # gates the initial all-engine barrier and delays kernel start.
    entry = nc.main_func.blocks[0]
    entry.instructions = [
        i for i in entry.instructions if not isinstance(i, mybir.InstMemset)
    ]

    nc.compile()
    return nc


def _get_nc():
    if "nc" not in _CACHE:
        _CACHE["nc"] = _build_nc()
    return _CACHE["nc"]


def build_mt(weights, lin_weights):
    """[P, KM*P] table; column block km holds M_km^T (matmul rhs layout)."""
    L = np.asarray(lin_weights, np.float32)
    w = np.asarray(weights, np.float32)
    a = np.arange(P)   # out index within chunk: a = j*16 + c'
    b = np.arange(P)   # in  index within chunk: b = i*16 + c
    mix = L[a[:, None] // TWO_R, b[None, :] // TWO_R] * (
        (a[:, None] % TWO_R) == (b[None, :] % TWO_R)
    ).astype(np.float32)
    mt = np.zeros((P, KM * P), np.float32)
    for km in range(KM):
        M = mix * w[km * P + b][None, :]       # [a, b]
        mt[:, km * P:(km + 1) * P] = M.T       # rhs[b, a] = M[a, b]
    return np.ascontiguousarray(mt)


def prep_in_maps(x, weights, lin_weights):
    xflat = np.asarray(x, np.float32).reshape(POS_TOTAL, N).astype(BF16)
    mt_host = build_mt(weights, lin_weights).astype(BF16)
    eye = np.eye(P, dtype=BF16)
    return [
        {"xs": np.ascontiguousarray(xflat[c * POS_PER_CORE:(c + 1) * POS_PER_CORE]),
         "mt": mt_host, "idt": eye}
        for c in range(N_CORES)
    ]


def kernel(x, weights, lin_weights):
    from concourse import bass_utils

    nc = _get_nc()
    in_maps = prep_in_maps(x, weights, lin_weights)
    res = bass_utils.run_bass_kernel_spmd(nc, in_maps, core_ids=list(range(N_CORES)))
    out = np.concatenate(
        [res.results[c]["out"].astype(np.float32) for c in range(N_CORES)], axis=0
    )
    return out.reshape(np.asarray(x).shape)
